# revision 20
# baseline (speedup 1.0000x reference)
"""Batch Gaussian rasterizer for TRN2 (Bass/Tile), SPMD over 8 NeuronCores.

Sharding: B=2 batch items x 4 image-slices (32 rows of 128 each core).
Inside a core: per-gaussian preprocess -> per-pixel-chunk (4x32 px) blending
with the transmittance cumprod done by tensor_tensor_scan along gaussians.
Gaussians are pre-sorted by depth on the host (cheap O(P) metadata op); all
heavy math runs on device.
"""
import sys
sys.path.insert(0, "/opt/trn_rl_repo")
import math
import numpy as np

import concourse.bass as bass
import concourse.bacc as bacc
import concourse.tile as tile
from concourse import mybir
from concourse.bass_utils import run_bass_kernel_spmd

F32 = mybir.dt.float32
F32R = mybir.dt.float32r
I32 = mybir.dt.int32
AF = mybir.ActivationFunctionType
OP = mybir.AluOpType

H_IMG = 128
W_IMG = 128
P_G = 1024          # gaussians per batch item
NCH = 8             # gaussian chunks of 128
ROWS_PER_CORE = 32
N_PIX_CH = 32       # pixel chunks per core (4 rows x 32 cols each)
TANFOV = 0.5
FX = W_IMG / (2.0 * TANFOV)   # 128
FY = H_IMG / (2.0 * TANFOV)   # 128
SCALE_MOD = 1.0
LN_THR = math.log(1.0 / 255.0)   # -5.5413
C0 = 0.28209479177387814
C1 = 0.4886025119029199
C2 = (1.0925484305920792, -1.0925484305920792, 0.31539156525252005,
      -1.0925484305920792, 0.5462742152960396)
C3 = (-0.5900435899266435, 2.890611442640554, -0.4570457994644658,
      0.3731763325901154, -0.4570457994644658, 1.445305721320277,
      -0.5900435899266435)

_CACHED = {}


def _build_nc():
    nc = bacc.Bacc(None, target_bir_lowering=False, debug=True)

    # ---------------- I/O ----------------
    g_mean_d = nc.declare_dram_parameter("g_mean", [128, NCH, 3], F32, isOutput=False)
    g_rot_d = nc.declare_dram_parameter("g_rot", [128, NCH, 4], F32, isOutput=False)
    g_scale_d = nc.declare_dram_parameter("g_scale", [128, NCH, 3], F32, isOutput=False)
    g_opac_d = nc.declare_dram_parameter("g_opac", [128, NCH], F32, isOutput=False)
    g_sh_d = nc.declare_dram_parameter("g_sh", [128, NCH, 48], F32, isOutput=False)
    consts_d = nc.declare_dram_parameter("consts", [1, 48], F32, isOutput=False)
    basis_d = nc.declare_dram_parameter("basis", [6, N_PIX_CH, 128], F32, isOutput=False)
    tgt_d = nc.declare_dram_parameter("tgt", [128, N_PIX_CH, 4], F32, isOutput=False)

    color_d = nc.declare_dram_parameter("color_o", [128, N_PIX_CH, 3], F32, isOutput=True)
    alpha_d = nc.declare_dram_parameter("alpha_o", [128, N_PIX_CH], F32, isOutput=True)
    est_d = nc.declare_dram_parameter("est_o", [4, P_G], F32, isOutput=True)
    radii_d = nc.declare_dram_parameter("radii_o", [128, NCH], F32, isOutput=True)

    with tile.TileContext(nc) as tc:
        with (
            tc.tile_pool(name="persist", bufs=1) as pp,
            tc.tile_pool(name="est_ps_pool", bufs=1, space="PSUM") as est_pool,
        ):
            # ------------- load inputs -------------
            g_mean = pp.tile([128, NCH, 3], F32)
            g_rot = pp.tile([128, NCH, 4], F32)
            g_scale = pp.tile([128, NCH, 3], F32)
            g_opac = pp.tile([128, NCH], F32)
            g_sh = pp.tile([128, NCH, 48], F32)
            consts = pp.tile([1, 48], F32)
            basis_sb = pp.tile([6, N_PIX_CH, 128], F32)
            tgt_sb = pp.tile([128, N_PIX_CH, 4], F32)
            nc.sync.dma_start(out=g_mean, in_=g_mean_d[:, :, :])
            nc.sync.dma_start(out=g_rot, in_=g_rot_d[:, :, :])
            nc.sync.dma_start(out=g_scale, in_=g_scale_d[:, :, :])
            nc.sync.dma_start(out=g_opac, in_=g_opac_d[:, :])
            nc.sync.dma_start(out=g_sh, in_=g_sh_d[:, :, :])
            nc.sync.dma_start(out=consts, in_=consts_d[:, :])
            nc.sync.dma_start(out=basis_sb, in_=basis_d[:, :, :])
            nc.sync.dma_start(out=tgt_sb, in_=tgt_d[:, :, :])

            est_ps = est_pool.tile([4, P_G], F32)

            with (
                tc.tile_pool(name="work", bufs=1) as wk,
                tc.tile_pool(name="pp_psum", bufs=1, space="PSUM") as ppp,
            ):
                # --------- broadcast consts to all 128 partitions ---------
                ones_row = wk.tile([1, 128], F32)
                nc.vector.memset(ones_row, 1.0)
                cb_ps = ppp.tile([128, 48], F32)
                nc.tensor.matmul(cb_ps, ones_row, consts, start=True, stop=True)
                cb = pp.tile([128, 48], F32)
                nc.scalar.copy(cb, cb_ps)

                def PJ(i, j):
                    return cb[:, 4 * i + j: 4 * i + j + 1]

                def VW(i, j):
                    return cb[:, 16 + 4 * i + j: 16 + 4 * i + j + 1]

                def CAM(i):
                    return cb[:, 32 + i: 32 + i + 1]

                CYC = cb[:, 41:42]      # cy
                CY2C = cb[:, 42:43]     # 2*cy
                CYSQ = cb[:, 43:44]     # cy^2

                # --------- identity matrices for PE transposes ---------
                r_i = wk.tile([128, 128], I32)
                c_i = wk.tile([128, 128], I32)
                nc.gpsimd.iota(r_i, pattern=[[0, 128]], channel_multiplier=1)
                nc.gpsimd.iota(c_i, pattern=[[1, 128]], channel_multiplier=0)
                ident_r = pp.tile([128, 128], F32R)
                nc.vector.tensor_tensor(out=ident_r, in0=r_i, in1=c_i, op=OP.is_equal)
                ident_f = pp.tile([128, 128], F32)
                nc.vector.tensor_tensor(out=ident_f, in0=r_i, in1=c_i, op=OP.is_equal)

                # --------- per-gaussian preprocess on [128, NCH] tiles ---------
                S = [128, NCH]

                _tn = [0]

                def T(name=None):
                    _tn[0] += 1
                    return wk.tile(S, F32, name=f"pp{_tn[0]}_{name or 't'}", uniquify=True)

                _tt_tog = [0]

                def tt(o, a, b, op):
                    if op in (OP.mult, OP.add, OP.subtract) and _tt_tog[0] % 2 == 0:
                        nc.gpsimd.tensor_tensor(out=o, in0=a, in1=b, op=op)
                    else:
                        nc.any.tensor_tensor(out=o, in0=a, in1=b, op=op)
                    _tt_tog[0] += 1
                    return o

                def ts(o, a, s1, op0, s2=None, op1=None):
                    if s2 is None:
                        nc.any.tensor_scalar(out=o, in0=a, scalar1=s1, scalar2=None, op0=op0)
                    else:
                        nc.any.tensor_scalar(out=o, in0=a, scalar1=s1, scalar2=s2,
                                             op0=op0, op1=op1)
                    return o

                def stt(o, a, s, b, op0, op1):
                    nc.vector.scalar_tensor_tensor(o, a, s, b, op0, op1)
                    return o

                def act(o, a, func, bias=0.0, scale=1.0):
                    nc.scalar.activation(o, a, func, bias=bias, scale=scale)
                    return o

                mx = g_mean[:, :, 0]
                my = g_mean[:, :, 1]
                mz = g_mean[:, :, 2]

                # hom_k = PJ[k,0]*mx + PJ[k,1]*my + PJ[k,2]*mz + PJ[k,3]
                def linrow(Mcol, k):
                    u1 = ts(T(), mx, Mcol(k, 0), OP.mult, Mcol(k, 3), OP.add)
                    u2 = stt(T(), my, Mcol(k, 1), u1, OP.mult, OP.add)
                    return stt(T(), mz, Mcol(k, 2), u2, OP.mult, OP.add)

                hom0 = linrow(PJ, 0)
                hom1 = linrow(PJ, 1)
                hom3 = linrow(PJ, 3)
                pv0 = linrow(VW, 0)
                pv1 = linrow(VW, 1)
                depth = linrow(VW, 2)

                hw = ts(T(), hom3, 1e-7, OP.add)
                pw = T("pw")
                nc.vector.reciprocal(pw, hw)
                t0 = tt(T(), hom0, pw, OP.mult)
                px = ts(T("px"), t0, float(W_IMG) * 0.5, OP.mult, (W_IMG - 1.0) * 0.5, OP.add)
                t1 = tt(T(), hom1, pw, OP.mult)
                py = ts(T("py"), t1, float(H_IMG) * 0.5, OP.mult, (H_IMG - 1.0) * 0.5, OP.add)

                m_if = ts(T(), depth, 0.2, OP.is_gt)

                # quaternion -> R (normalized)
                qq = wk.tile([128, NCH, 4], F32)
                nc.any.tensor_tensor(out=qq, in0=g_rot, in1=g_rot, op=OP.mult)
                n2 = tt(T(), qq[:, :, 0], qq[:, :, 1], OP.add)
                n2 = tt(T(), n2, qq[:, :, 2], OP.add)
                n2 = tt(T(), n2, qq[:, :, 3], OP.add)
                rn2 = T()
                nc.vector.reciprocal(rn2, n2)
                rno = act(T(), rn2, AF.Sqrt)
                qr = tt(T(), g_rot[:, :, 0], rno, OP.mult)
                qx = tt(T(), g_rot[:, :, 1], rno, OP.mult)
                qy = tt(T(), g_rot[:, :, 2], rno, OP.mult)
                qz = tt(T(), g_rot[:, :, 3], rno, OP.mult)

                xx = tt(T(), qx, qx, OP.mult)
                yy = tt(T(), qy, qy, OP.mult)
                zz = tt(T(), qz, qz, OP.mult)
                xy = tt(T(), qx, qy, OP.mult)
                xz = tt(T(), qx, qz, OP.mult)
                yz = tt(T(), qy, qz, OP.mult)
                rx = tt(T(), qr, qx, OP.mult)
                ry = tt(T(), qr, qy, OP.mult)
                rz = tt(T(), qr, qz, OP.mult)

                def r_diag(a, b):
                    s = tt(T(), a, b, OP.add)
                    return ts(T(), s, -2.0, OP.mult, 1.0, OP.add)

                def r_off(a, b, sgn):
                    s = tt(T(), a, b, OP.add if sgn < 0 else OP.subtract)
                    # sgn>0: 2*(a - b); sgn<0: 2*(a + b) ... handled by caller
                    return ts(T(), s, 2.0, OP.mult)

                R00 = r_diag(yy, zz)
                R11 = r_diag(xx, zz)
                R22 = r_diag(xx, yy)
                # R01 = 2(xy - rz), R02 = 2(xz + ry)
                R01 = r_off(xy, rz, +1)
                R02 = r_off(xz, ry, -1)
                R10 = r_off(xy, rz, -1)
                R12 = r_off(yz, rx, +1)
                R20 = r_off(xz, ry, +1)
                R21 = r_off(yz, rx, -1)
                R = [[R00, R01, R02], [R10, R11, R12], [R20, R21, R22]]

                # M[i][j] = R[i][j] * scale_j * SCALE_MOD
                M = [[tt(T(), R[i][j], g_scale[:, :, j], OP.mult) for j in range(3)]
                     for i in range(3)]

                # Sigma = M @ M.T (symmetric, need upper entries)
                def dot3(arow, brow):
                    p0 = tt(T(), arow[0], brow[0], OP.mult)
                    p1 = tt(T(), arow[1], brow[1], OP.mult)
                    p01 = tt(T(), p0, p1, OP.add)
                    p2 = tt(T(), arow[2], brow[2], OP.mult)
                    return tt(T(), p01, p2, OP.add)

                S00 = dot3(M[0], M[0])
                S01 = dot3(M[0], M[1])
                S02 = dot3(M[0], M[2])
                S11 = dot3(M[1], M[1])
                S12 = dot3(M[1], M[2])
                S22 = dot3(M[2], M[2])
                Sig = [[S00, S01, S02], [S01, S11, S12], [S02, S12, S22]]

                # EWA Jacobian
                invz = T()
                nc.vector.reciprocal(invz, depth)
                tx0 = tt(T(), pv0, invz, OP.mult)
                txc = ts(T(), tx0, 1.3 * TANFOV, OP.min, -1.3 * TANFOV, OP.max)
                txc = tt(T(), txc, depth, OP.mult)
                ty0 = tt(T(), pv1, invz, OP.mult)
                tyc = ts(T(), ty0, 1.3 * TANFOV, OP.min, -1.3 * TANFOV, OP.max)
                tyc = tt(T(), tyc, depth, OP.mult)
                iz2 = tt(T(), invz, invz, OP.mult)
                J00 = ts(T(), invz, FX, OP.mult)
                J11 = ts(T(), invz, FY, OP.mult)
                t2_ = tt(T(), txc, iz2, OP.mult)
                J02 = ts(T(), t2_, -FX, OP.mult)
                t3_ = tt(T(), tyc, iz2, OP.mult)
                J12 = ts(T(), t3_, -FY, OP.mult)

                # T = J @ view[:3,:3]  (J01 = J10 = 0)
                Trow = [[None] * 3, [None] * 3]
                for j in range(3):
                    a = ts(T(), J00, VW(0, j), OP.mult)
                    Trow[0][j] = stt(T(), J02, VW(2, j), a, OP.mult, OP.add)
                    b = ts(T(), J11, VW(1, j), OP.mult)
                    Trow[1][j] = stt(T(), J12, VW(2, j), b, OP.mult, OP.add)

                # cov2d = T Sigma T^T
                U = [[dot3(Trow[i], [Sig[0][k], Sig[1][k], Sig[2][k]]) for k in range(3)]
                     for i in range(2)]
                c00 = dot3(U[0], Trow[0])
                c01 = dot3(U[0], Trow[1])
                c11 = dot3(U[1], Trow[1])
                c00 = ts(T(), c00, 0.3, OP.add)
                c11 = ts(T(), c11, 0.3, OP.add)

                d1 = tt(T(), c00, c11, OP.mult)
                d2 = tt(T(), c01, c01, OP.mult)
                det = tt(T(), d1, d2, OP.subtract)
                md = ts(T(), det, 1e-12, OP.is_gt)
                valid = tt(T(), md, m_if, OP.mult)
                nmd = ts(T(), md, -1.0, OP.mult, 1.0, OP.add)
                dm = tt(T(), det, md, OP.mult)
                dsafe = tt(T(), dm, nmd, OP.add)
                invd = T()
                nc.vector.reciprocal(invd, dsafe)

                con_a = tt(T(), c11, invd, OP.mult)
                con_c = tt(T(), c00, invd, OP.mult)
                Fco = tt(T("Fco"), c01, invd, OP.mult)   # = -conic_b

                # radii
                mid = tt(T(), c00, c11, OP.add)
                mid = ts(T(), mid, 0.5, OP.mult)
                m2 = tt(T(), mid, mid, OP.mult)
                dd = tt(T(), m2, det, OP.subtract)
                dd = ts(T(), dd, 0.1, OP.max)
                sq = act(T(), dd, AF.Sqrt)
                lam = tt(T(), mid, sq, OP.add)
                rl = act(T(), lam, AF.Sqrt)
                r3 = ts(T(), rl, 3.0, OP.mult)
                ri = wk.tile(S, I32)
                nc.any.tensor_copy(out=ri, in_=r3)
                rf = T()
                nc.any.tensor_copy(out=rf, in_=ri)
                gt = tt(T(), r3, rf, OP.is_gt)
                ce = tt(T(), rf, gt, OP.add)
                radii_sb = pp.tile([128, NCH], F32)
                tt(radii_sb, ce, valid, OP.mult)
                nc.sync.dma_start(out=radii_d[:, :], in_=radii_sb)

                # SH colors
                dxg = ts(T(), mx, CAM(0), OP.subtract)
                dyg = ts(T(), my, CAM(1), OP.subtract)
                dzg = ts(T(), mz, CAM(2), OP.subtract)
                nn0 = tt(T(), dxg, dxg, OP.mult)
                nn1 = tt(T(), dyg, dyg, OP.mult)
                nn = tt(T(), nn0, nn1, OP.add)
                nn2 = tt(T(), dzg, dzg, OP.mult)
                nn = tt(T(), nn, nn2, OP.add)
                rnn = T()
                nc.vector.reciprocal(rnn, nn)
                rno2 = act(T(), rnn, AF.Sqrt)
                sx = tt(T(), dxg, rno2, OP.mult)
                sy = tt(T(), dyg, rno2, OP.mult)
                sz = tt(T(), dzg, rno2, OP.mult)
                sxx = tt(T(), sx, sx, OP.mult)
                syy = tt(T(), sy, sy, OP.mult)
                szz = tt(T(), sz, sz, OP.mult)
                sxy = tt(T(), sx, sy, OP.mult)
                syz = tt(T(), sy, sz, OP.mult)
                sxz = tt(T(), sx, sz, OP.mult)

                B = wk.tile([128, NCH, 16], F32)
                nc.any.memset(B[:, :, 0], C0)
                ts(B[:, :, 1], sy, -C1, OP.mult)
                ts(B[:, :, 2], sz, C1, OP.mult)
                ts(B[:, :, 3], sx, -C1, OP.mult)
                ts(B[:, :, 4], sxy, C2[0], OP.mult)
                ts(B[:, :, 5], syz, C2[1], OP.mult)
                u_ = tt(T("xxyy"), sxx, syy, OP.add)
                t6 = stt(T(), szz, 2.0, u_, OP.mult, OP.subtract)
                ts(B[:, :, 6], t6, C2[2], OP.mult)
                ts(B[:, :, 7], sxz, C2[3], OP.mult)
                df_ = tt(T("xxmyy"), sxx, syy, OP.subtract)
                ts(B[:, :, 8], df_, C2[4], OP.mult)
                t9 = stt(T(), sxx, 3.0, syy, OP.mult, OP.subtract)
                t9b = tt(T(), sy, t9, OP.mult)
                ts(B[:, :, 9], t9b, C3[0], OP.mult)
                t10 = tt(T(), sxy, sz, OP.mult)
                ts(B[:, :, 10], t10, C3[1], OP.mult)
                t11 = stt(T("zz4mu"), szz, 4.0, u_, OP.mult, OP.subtract)
                t11b = tt(T(), sy, t11, OP.mult)
                ts(B[:, :, 11], t11b, C3[2], OP.mult)
                v2_ = ts(T(), szz, 2.0, OP.mult)
                t12 = stt(T(), u_, -3.0, v2_, OP.mult, OP.add)
                t12b = tt(T(), sz, t12, OP.mult)
                ts(B[:, :, 12], t12b, C3[3], OP.mult)
                t13b = tt(T(), sx, t11, OP.mult)
                ts(B[:, :, 13], t13b, C3[4], OP.mult)
                t14b = tt(T(), sz, df_, OP.mult)
                ts(B[:, :, 14], t14b, C3[5], OP.mult)
                t15 = stt(T(), syy, 3.0, sxx, OP.mult, OP.subtract)
                t15b = tt(T(), sx, t15, OP.mult)
                ts(B[:, :, 15], t15b, -C3[6], OP.mult)

                colf = wk.tile([128, NCH, 4], F32)
                nc.vector.memset(colf, 0.0)
                prod = wk.tile([128, NCH, 16], F32)
                for ch in range(3):
                    sh_c = g_sh.rearrange("p c (k t) -> p c k t", t=3)[:, :, :, ch]
                    nc.any.tensor_tensor(out=prod, in0=B, in1=sh_c, op=OP.mult)
                    red = wk.tile([128, NCH], F32, name=f"red{ch}", uniquify=True)
                    nc.vector.tensor_reduce(red, prod, mybir.AxisListType.X, OP.add)
                    ts(colf[:, :, ch], red, 0.5, OP.add, 0.0, OP.max)


                col_r = pp.tile([128, NCH, 4], F32R)
                nc.vector.tensor_copy(out=col_r, in_=colf)

                # ---- monomial coefficients (global coords) ----
                lnop = act(T(), g_opac, AF.Ln)
                pen = ts(T(), valid, 1e30, OP.mult, -1e30, OP.add)
                lnC = tt(T(), lnop, pen, OP.add)

                Dco = ts(T("Dco"), con_a, -0.5, OP.mult)
                Eco = ts(T("Eco"), con_c, -0.5, OP.mult)
                a_px = tt(T(), con_a, px, OP.mult)
                f_py = tt(T(), Fco, py, OP.mult)
                Aco = tt(T("Aco"), a_px, f_py, OP.subtract)
                c_py = tt(T(), con_c, py, OP.mult)
                f_px = tt(T(), Fco, px, OP.mult)
                Bco = tt(T("Bco"), c_py, f_px, OP.subtract)
                px2 = tt(T(), px, px, OP.mult)
                py2 = tt(T(), py, py, OP.mult)
                pxy = tt(T(), px, py, OP.mult)
                cd = tt(T(), Dco, px2, OP.mult)
                ce2 = tt(T(), Eco, py2, OP.mult)
                cde = tt(T(), cd, ce2, OP.add)
                cf = tt(T(), Fco, pxy, OP.mult)
                Cco = tt(T(), cde, cf, OP.add)
                Cb = tt(T("Cb"), Cco, lnC, OP.add)

                Fcy = ts(T("Fcy"), Fco, CYC, OP.mult)

                # variant rows into fblock [128, NCH, 24]
                fblock = pp.tile([128, NCH, 24], F32)
                for v in range(4):
                    cx = 32.0 * v + 15.5
                    base = 6 * v
                    # C'v
                    t1v = stt(T(), Aco, cx, Cb, OP.mult, OP.add)
                    t2v = stt(T(), Bco, CYC, t1v, OP.mult, OP.add)
                    t3v = stt(T(), Dco, cx * cx, t2v, OP.mult, OP.add)
                    t4v = stt(T(), Eco, CYSQ, t3v, OP.mult, OP.add)
                    stt(fblock[:, :, base + 0], Fcy, cx, t4v, OP.mult, OP.add)
                    # A'v = Aco + 2cx*D + Fcy
                    tav = stt(T(), Dco, 2.0 * cx, Aco, OP.mult, OP.add)
                    tt(fblock[:, :, base + 1], tav, Fcy, OP.add)
                    # B'v = Bco + 2cy*E + cx*F
                    tbv = stt(T(), Eco, CY2C, Bco, OP.mult, OP.add)
                    stt(fblock[:, :, base + 2], Fco, cx, tbv, OP.mult, OP.add)
                    nc.any.tensor_copy(out=fblock[:, :, base + 3], in_=Dco)
                    nc.any.tensor_copy(out=fblock[:, :, base + 4], in_=Eco)
                    nc.any.tensor_copy(out=fblock[:, :, base + 5], in_=Fco)

                # transpose coef then DMA-rebase each variant to partition 0
                coef_v = [pp.tile([6, P_G], F32, name=f"coef{v}", uniquify=True)
                          for v in range(4)]
                ct_all = wk.tile([24, P_G], F32)
                for gc in range(NCH):
                    ct_ps = ppp.tile([24, 128], F32, name="ct_ps", uniquify=True)
                    nc.tensor.transpose(ct_ps, fblock[:, gc, :], ident_f)
                    nc.scalar.copy(ct_all[:, gc * 128:(gc + 1) * 128], ct_ps)
                for v in range(4):
                    nc.sync.dma_start(out=coef_v[v][:, :],
                                      in_=ct_all[6 * v:6 * v + 6, :])

                tgt_r = pp.tile([128, N_PIX_CH, 4], F32R)
                nc.vector.tensor_copy(out=tgt_r, in_=tgt_sb)

            # ================= main per-pixel-chunk loop =================
            with (
                tc.tile_pool(name="loop_sb", bufs=3) as lb,
                tc.tile_pool(name="P_pool", bufs=1, space="PSUM") as P_pool,
                tc.tile_pool(name="wT_pool", bufs=1, space="PSUM") as wT_pool,
                tc.tile_pool(name="col_pool", bufs=2, space="PSUM") as col_pool,
            ):
                alpha_sb = pp.tile([128, N_PIX_CH], F32)
                est_sb = pp.tile([4, P_G], F32)
                color_all = pp.tile([128, N_PIX_CH, 3], F32)
                sign_b = pp.tile([128, 1], F32)
                nc.vector.memset(sign_b, 1.0 / 255.0)

                for c in range(N_PIX_CH):
                    v = c % 4
                    P_ps = P_pool.tile([128, P_G], F32, name="P_ps", uniquify=True)
                    for h in range(2):
                        nc.tensor.matmul(
                            P_ps[:, h * 512:(h + 1) * 512],
                            basis_sb[:, c, :],
                            coef_v[v][:, h * 512:(h + 1) * 512],
                            start=True, stop=True,
                        )
                    e = lb.tile([128, P_G], F32, name="e", uniquify=True)
                    nc.scalar.activation(e, P_ps, AF.Exp)
                    nm = lb.tile([128, P_G], F32, name="nm", uniquify=True)
                    nc.scalar.activation(nm, e, AF.Sign, scale=-1.0, bias=sign_b)
                    z = lb.tile([128, P_G], F32, name="z", uniquify=True)
                    nc.vector.tensor_scalar(out=z, in0=e, scalar1=-1.0, scalar2=1.0,
                                            op0=OP.mult, op1=OP.add)
                    om = lb.tile([128, P_G + 1], F32, name="om", uniquify=True)
                    nc.gpsimd.memset(om[:, 0:1], 1.0)
                    nc.vector.scalar_tensor_tensor(
                        om[:, 1:P_G + 1], z, 0.01, nm, OP.max, OP.max)
                    Texc = lb.tile([128, P_G + 1], F32, name="Texc", uniquify=True)
                    nc.vector.tensor_tensor_scan(
                        Texc, om, om, 1.0, OP.mult, OP.bypass)
                    w_r = lb.tile([128, P_G], F32R, name="w_r", uniquify=True)
                    nc.vector.tensor_tensor(
                        out=w_r, in0=Texc[:, 0:P_G], in1=Texc[:, 1:P_G + 1],
                        op=OP.subtract)
                    # alpha image column: 1 - T_final
                    nc.vector.tensor_scalar(
                        out=alpha_sb[:, c:c + 1], in0=Texc[:, P_G:P_G + 1],
                        scalar1=-1.0, scalar2=1.0, op0=OP.mult, op1=OP.add)

                    # est accumulation: est += tgt_c^T-style (lhsT=[128px,4])
                    for h in range(2):
                        nc.tensor.matmul(
                            est_ps[:, h * 512:(h + 1) * 512],
                            tgt_r[:, c, :],
                            w_r[:, h * 512:(h + 1) * 512],
                            start=(c == 0), stop=(c == N_PIX_CH - 1),
                        )

                    # transpose w for the color contraction
                    wT_ps = wT_pool.tile([128, P_G], F32R, name="wT_ps", uniquify=True)
                    for t in range(NCH):
                        nc.tensor.transpose(
                            wT_ps[:, t * 128:(t + 1) * 128],
                            w_r[:, t * 128:(t + 1) * 128], ident_r)
                    wT_sb = lb.tile([128, P_G], F32R, name="wT_sb", uniquify=True)
                    nc.scalar.copy(wT_sb, wT_ps)

                    col_ps = col_pool.tile([128, 4], F32, name="col_ps", uniquify=True)
                    for t in range(NCH):
                        nc.tensor.matmul(
                            col_ps,
                            wT_sb[:, t * 128:(t + 1) * 128],
                            col_r[:, t, :],
                            start=(t == 0), stop=(t == NCH - 1),
                        )
                    for ch in range(3):
                        nc.vector.scalar_tensor_tensor(
                            color_all[:, c, ch:ch + 1], Texc[:, P_G:P_G + 1],
                            cb[:, 35 + ch:36 + ch], col_ps[:, ch:ch + 1],
                            OP.mult, OP.add)

                nc.scalar.copy(est_sb, est_ps)
                nc.sync.dma_start(out=est_d[:, :], in_=est_sb)
                nc.sync.dma_start(out=alpha_d[:, :], in_=alpha_sb)
                nc.sync.dma_start(out=color_d[:, :, :], in_=color_all)

    nc.compile()
    return nc


def _host_static(y0):
    """Per-core basis [6,32,128] for image rows y0..y0+31 (4x32 px chunks)."""
    basis = np.zeros((6, N_PIX_CH, 128), np.float32)
    cy = y0 + 15.5
    for c in range(N_PIX_CH):
        r, v = c // 4, c % 4
        p = np.arange(128)
        xg = 32 * v + (p % 32)
        yg = y0 + 4 * r + (p // 32)
        xp = xg - (32 * v + 15.5)
        yp = yg - cy
        basis[0, c] = 1.0
        basis[1, c] = xp
        basis[2, c] = yp
        basis[3, c] = xp * xp
        basis[4, c] = yp * yp
        basis[5, c] = xp * yp
    return basis


def _pix_maps(y0):
    """chunk,partition -> (y_global_row_offset_in_core, x)"""
    cs, ps = np.meshgrid(np.arange(N_PIX_CH), np.arange(128), indexing="ij")
    r, v = cs // 4, cs % 4
    yy = 4 * r + (ps // 32)          # row within core slice
    xx = 32 * v + (ps % 32)
    return yy, xx


def _build_runner(nc, n_cores=8):
    import jax
    import numpy as _np
    from jax.sharding import Mesh, PartitionSpec
    from jax.experimental.shard_map import shard_map
    from concourse import bass2jax, mybir as _mb
    bass2jax.install_neuronx_cc_hook()

    partition_name = nc.partition_id_tensor.name if nc.partition_id_tensor else None
    in_names, out_names, out_avals, zero_outs = [], [], [], []
    for alloc in nc.m.functions[0].allocations:
        if not isinstance(alloc, _mb.MemoryLocationSet):
            continue
        name = alloc.memorylocations[0].name
        if alloc.kind == "ExternalInput":
            if name != partition_name:
                in_names.append(name)
        elif alloc.kind == "ExternalOutput":
            out_names.append(name)
            shape = tuple(alloc.tensor_shape)
            dtype = _mb.dt.np(alloc.dtype)
            out_avals.append(jax.core.ShapedArray(shape, dtype))
            zero_outs.append(_np.zeros(shape, dtype))
    n_params = len(in_names)
    n_outs = len(out_avals)
    all_in_names = list(in_names) + list(out_names)
    if partition_name is not None:
        all_in_names.append(partition_name)
    donate = tuple(range(n_params, n_params + n_outs))

    def _body(*args):
        operands = list(args)
        if partition_name is not None:
            operands.append(bass2jax.partition_id_tensor())
        outs = bass2jax._bass_exec_p.bind(
            *operands,
            out_avals=tuple(out_avals),
            in_names=tuple(all_in_names),
            out_names=tuple(out_names),
            lowering_input_output_aliases=(),
            sim_require_finite=True,
            sim_require_nnan=True,
            nc=nc,
        )
        return tuple(outs)

    devices = jax.devices()[:n_cores]
    mesh = Mesh(_np.asarray(devices), ("core",))
    in_specs = (PartitionSpec("core"),) * (n_params + n_outs)
    out_specs = (PartitionSpec("core"),) * len(out_names)
    sharded = jax.jit(
        shard_map(_body, mesh=mesh, in_specs=in_specs, out_specs=out_specs,
                  check_rep=False),
        keep_unused=True,
    )

    dbg_name = nc.dbg_addr.name if nc.dbg_addr is not None else None

    def run(in_maps):
        if dbg_name is not None:
            in_maps = [{**m, dbg_name: _np.zeros((1, 2), _np.uint32)} for m in in_maps]
        concat_in = [
            _np.concatenate([_np.asarray(in_maps[c][nm]) for c in range(n_cores)], axis=0)
            for nm in in_names
        ]
        if "dev_zeros" not in _CACHED:
            _CACHED["dev_zeros"] = [
                _np.zeros((n_cores * z.shape[0], *z.shape[1:]), z.dtype)
                for z in zero_outs
            ]
        out_arrs = sharded(*concat_in, *_CACHED["dev_zeros"])
        return [
            {name: _np.asarray(out_arrs[i]).reshape(n_cores, *out_avals[i].shape)[c]
             for i, name in enumerate(out_names)}
            for c in range(n_cores)
        ]

    return run


def _run_cached(nc, in_maps):
    if "runner" not in _CACHED:
        _CACHED["runner"] = _build_runner(nc)
    return _CACHED["runner"](in_maps)


def kernel(means3D, sh, opacities, scales, rotations, target_image, bg,
           viewmatrix, projmatrix, campos):
    if "nc" not in _CACHED:
        _CACHED["nc"] = _build_nc()
    nc = _CACHED["nc"]

    means3D = np.asarray(means3D, np.float32)
    sh = np.asarray(sh, np.float32)
    opacities = np.asarray(opacities, np.float32)
    scales = np.asarray(scales, np.float32)
    rotations = np.asarray(rotations, np.float32)
    target_image = np.asarray(target_image, np.float32)
    bg = np.asarray(bg, np.float32)
    V = np.asarray(viewmatrix, np.float32)
    PJ = np.asarray(projmatrix, np.float32)
    campos_np = np.asarray(campos, np.float32)

    Bb = means3D.shape[0]
    orders, invs = [], []
    for b in range(Bb):
        depth = (means3D[b] @ V.T[:3, 2] + V.T[3, 2]).astype(np.float32)
        order = np.argsort(depth, kind="stable")
        orders.append(order)
        invs.append(np.argsort(order, kind="stable"))

    def glayout(x):
        # [P, ...] sorted -> [128, 8, ...]
        return np.ascontiguousarray(
            x.reshape(NCH, 128, *x.shape[1:]).transpose(1, 0, *range(2, x.ndim + 1)))

    in_maps = []
    for core in range(8):
        b, sl = core // 4, core % 4
        y0 = ROWS_PER_CORE * sl
        o = orders[b]
        consts = np.zeros((1, 48), np.float32)
        consts[0, 0:16] = PJ.reshape(-1)
        consts[0, 16:32] = V.reshape(-1)
        consts[0, 32:35] = campos_np
        consts[0, 35:38] = bg[b]
        cy = y0 + 15.5
        consts[0, 41] = cy
        consts[0, 42] = 2.0 * cy
        consts[0, 43] = cy * cy

        tgt = np.zeros((128, N_PIX_CH, 4), np.float32)
        yy, xx = _pix_maps(y0)
        timg = target_image[b]  # [3, H, W]
        for ch in range(3):
            tgt[:, :, ch] = timg[ch, y0 + yy, xx].T
        tgt[:, :, 3] = 1.0

        in_maps.append({
            "g_mean": glayout(means3D[b][o]),
            "g_rot": glayout(rotations[b][o]),
            "g_scale": glayout(scales[b][o]),
            "g_opac": glayout(opacities[b][o, 0]),
            "g_sh": glayout(sh[b][o].reshape(P_G, 48)),
            "consts": consts,
            "basis": _host_static(y0),
            "tgt": tgt,
        })

    res = _run_cached(nc, in_maps)

    color = np.zeros((Bb, 3, H_IMG, W_IMG), np.float32)
    alpha_img = np.zeros((Bb, 1, H_IMG, W_IMG), np.float32)
    est_color = np.zeros((Bb, P_G, 3), np.float32)
    est_weight = np.zeros((Bb, P_G, 1), np.float32)
    radii = np.zeros((Bb, P_G), np.float32)

    for core in range(8):
        b, sl = core // 4, core % 4
        y0 = ROWS_PER_CORE * sl
        r = res[core]
        yy, xx = _pix_maps(y0)
        for ch in range(3):
            color[b, ch, y0 + yy, xx] = r["color_o"][:, :, ch].T
        alpha_img[b, 0, y0 + yy, xx] = r["alpha_o"].T
        est = r["est_o"]  # [4, P] sorted order
        est_color[b] += est[0:3].T[invs[b]]
        est_weight[b, :, 0] += est[3][invs[b]]
        if sl == 0:
            radii[b] = r["radii_o"].T.reshape(-1)[invs[b]]

    return color, alpha_img, est_color, est_weight, radii
